# revision 1
# baseline (speedup 1.0000x reference)
"""Trainium2 Bass kernel for nn_Decoder_3289944948995 (GNN message-passing decoder).

Reference computation (per edge e):
    z   = concat(z_drug[row[e]], z_reaction[col[e]])          # [2H] = [1024]
    h   = relu(W1 @ z + b1)                                   # [512]
    out = W2 @ h + b2                                         # scalar

Algebraic restructure: W1 @ concat(zd, zr) = W1d @ zd + W1r @ zr, so
    A = z_drug     @ W1[:, :512].T + b1     # [2000, 512]   (node table)
    B = z_reaction @ W1[:, 512:].T          # [10000, 512]  (node table)
    out[e] = w2 . relu(A[row[e]] + B[col[e]]) + b2

This turns 420 GFLOP of per-edge matmul into ~6 GFLOP of per-node precompute
plus per-edge gather + add + relu + matvec.

Device schedule (identical SPMD program on 8 cores; core i owns edges
[i*50000, (i+1)*50000)):
  Phase 1: precompute A/B on the PE from host-transposed fp16 z-tables,
           write fp16 row-major tables to DRAM scratch.
  Phase 2: per 2048-edge tile: transposed dma_gather of A[row]/B[col]
           ([128 h-part, 4 h-chunk, 2048 edge] layout), DVE add, relu
           (split DVE/ACT), PE matvec with w2 (PSUM accumulate), +b2,
           DMA out.

Host side only reshapes/casts/shards inputs and concatenates outputs.
"""

import numpy as np

H = 512
N_DRUG, N_REACTION, N_EDGES = 2000, 10000, 400000
N_CORES = 8
E_CORE = N_EDGES // N_CORES          # 50000 edges per core
ET = 2048                            # edges per gather tile
NT = -(-E_CORE // ET)                # 25 tiles
E_PAD = NT * ET                      # 51200 (padded with index 0)
IDX_COLS = ET // 16                  # 128 idx columns per tile
A_ROWS, B_ROWS = 2048, 10240         # node tables padded to 128 multiple
ZBLK = 1024                          # precompute node-block
KC = H // 128                        # 4 contraction chunks of 128

_CACHE = {}


def _build_nc():
    import concourse.bacc as bacc
    import concourse.mybir as mybir
    import concourse.tile as tile
    from concourse import library_config
    from concourse.bass import ts

    dt = mybir.dt
    nc = bacc.Bacc(None, target_bir_lowering=False)

    zdT = nc.dram_tensor("zdT", [H, A_ROWS], dt.float16, kind="ExternalInput")
    zrT = nc.dram_tensor("zrT", [H, B_ROWS], dt.float16, kind="ExternalInput")
    w1dT = nc.dram_tensor("w1dT", [H, H], dt.float16, kind="ExternalInput")
    w1rT = nc.dram_tensor("w1rT", [H, H], dt.float16, kind="ExternalInput")
    b1f = nc.dram_tensor("b1f", [128, H], dt.float32, kind="ExternalInput")
    w2T = nc.dram_tensor("w2T", [128, KC * 32], dt.float16, kind="ExternalInput")
    b2v = nc.dram_tensor("b2v", [128, 1], dt.float32, kind="ExternalInput")
    rowidx = nc.dram_tensor(
        "rowidx", [128, NT * IDX_COLS], dt.int16, kind="ExternalInput"
    )
    colidx = nc.dram_tensor(
        "colidx", [128, NT * IDX_COLS], dt.int16, kind="ExternalInput"
    )
    out = nc.dram_tensor("out", [E_PAD], dt.float32, kind="ExternalOutput")

    with tile.TileContext(nc) as tc:
        with (
            tc.tile_pool(name="const", bufs=1) as cpool,
            tc.tile_pool(name="z", bufs=2) as zpool,
            tc.tile_pool(name="o1", bufs=3) as opool,
            tc.tile_pool(name="g", bufs=2) as gpool,
            tc.tile_pool(name="fin", bufs=2) as fpool,
            tc.tile_pool(name="ps1", bufs=4, space="PSUM") as ps1,
            tc.tile_pool(name="ps2", bufs=4, space="PSUM") as ps2,
            tc.tile_pool(name="dram", bufs=1, space="DRAM") as dpool,
        ):
            # dma_gather (DMAGatherAnt) lives in the 'mlp' GPSIMD library
            nc.gpsimd.load_library(library_config.mlp)

            # ---- constant / index preload ----
            w1d_sb = cpool.tile([128, KC, H], dt.float16)
            nc.sync.dma_start(
                out=w1d_sb[:], in_=w1dT[:, :].rearrange("(c p) o -> p c o", p=128)
            )
            w1r_sb = cpool.tile([128, KC, H], dt.float16)
            nc.sync.dma_start(
                out=w1r_sb[:], in_=w1rT[:, :].rearrange("(c p) o -> p c o", p=128)
            )
            b1_sb = cpool.tile([128, H], dt.float32)
            nc.sync.dma_start(out=b1_sb[:], in_=b1f[:, :])
            w2_sb = cpool.tile([128, KC, 32], dt.float16)
            nc.sync.dma_start(
                out=w2_sb[:], in_=w2T[:, :].rearrange("p (c m) -> p c m", m=32)
            )
            b2_sb = cpool.tile([128, 1], dt.float32)
            nc.sync.dma_start(out=b2_sb[:], in_=b2v[:, :])
            row_sb = cpool.tile([128, NT * IDX_COLS], dt.int16)
            nc.sync.dma_start(out=row_sb[:], in_=rowidx[:, :])
            col_sb = cpool.tile([128, NT * IDX_COLS], dt.int16)
            nc.sync.dma_start(out=col_sb[:], in_=colidx[:, :])

            A_t = dpool.tile([A_ROWS, H], dt.float16, tag="A")
            B_t = dpool.tile([B_ROWS, H], dt.float16, tag="B")

            # ---- phase 1: node tables A = zd@W1d.T + b1, B = zr@W1r.T ----
            def precompute(zT_handle, w1_sb, table, n_rows, add_b1):
                z_ap = zT_handle[:, :].rearrange(
                    "(c p) (b n) -> b p c n", p=128, n=ZBLK
                )
                for b in range(n_rows // ZBLK):
                    zt = zpool.tile([128, KC, ZBLK], dt.float16, tag="zt")
                    nc.sync.dma_start(out=zt[:], in_=z_ap[b])
                    for nt_ in range(ZBLK // 128):
                        psum = ps1.tile([128, H], dt.float32, tag="ps1")
                        for c in range(KC):
                            nc.tensor.matmul(
                                out=psum[:],
                                lhsT=zt[:, c, ts(nt_, 128)],
                                rhs=w1_sb[:, c, :],
                                start=(c == 0),
                                stop=(c == KC - 1),
                            )
                        osb = opool.tile([128, H], dt.float16, tag="osb")
                        if add_b1:
                            nc.vector.tensor_add(out=osb[:], in0=psum[:], in1=b1_sb[:])
                        else:
                            nc.scalar.copy(out=osb[:], in_=psum[:])
                        r0 = b * ZBLK + nt_ * 128
                        nc.sync.dma_start(out=table[r0 : r0 + 128, :], in_=osb[:])

            precompute(zdT, w1d_sb, A_t, A_ROWS, add_b1=True)
            precompute(zrT, w1r_sb, B_t, B_ROWS, add_b1=False)

            # ---- phase 2: per-edge gather + add + relu + w2 matvec ----
            out_ap = out[:].rearrange("(t g n) -> t g n", g=4, n=512)
            for t in range(NT):
                ag = gpool.tile([128, KC, ET], dt.float16, tag="ag")
                bg = gpool.tile([128, KC, ET], dt.float16, tag="bg")
                nc.gpsimd.dma_gather(
                    out_ap=ag[:],
                    in_ap=A_t[:, :],
                    idxs_ap=row_sb[:, ts(t, IDX_COLS)],
                    num_idxs=ET,
                    num_idxs_reg=ET,
                    elem_size=H,
                    transpose=True,
                    single_packet=False,
                )
                nc.gpsimd.dma_gather(
                    out_ap=bg[:],
                    in_ap=B_t[:, :],
                    idxs_ap=col_sb[:, ts(t, IDX_COLS)],
                    num_idxs=ET,
                    num_idxs_reg=ET,
                    elem_size=H,
                    transpose=True,
                    single_packet=False,
                )
                tt = gpool.tile([128, KC, ET], dt.float16, tag="tt")
                nc.vector.tensor_add(out=tt[:], in0=ag[:], in1=bg[:])
                if t % 3 == 0:
                    nc.vector.tensor_scalar_max(out=tt[:], in0=tt[:], scalar1=0.0)
                else:
                    nc.scalar.activation(
                        out=tt[:], in_=tt[:], func=mybir.ActivationFunctionType.Relu
                    )
                psum = ps2.tile([128, 512], dt.float32, tag="ps2")
                for g in range(4):
                    # w2 chunk replicated over 32 PE columns: group g fills
                    # psum partitions [g*32, (g+1)*32) with identical rows, so
                    # the whole bank is written (no uninitialized reads) and
                    # the finisher is a single full-tile op. PE output base
                    # partition must be 32-aligned; explicit tile_position
                    # because base_partition() rejects 96.
                    for c in range(KC):
                        nc.tensor.matmul(
                            out=psum[g * 32 : (g + 1) * 32, :],
                            lhsT=w2_sb[:, c, :],
                            rhs=tt[:, c, ts(g, 512)],
                            start=(c == 0),
                            stop=(c == KC - 1),
                            tile_position=(0, g * 32),
                        )
                fsb = fpool.tile([128, 512], dt.float32, tag="fout")
                nc.vector.tensor_scalar_add(
                    out=fsb[:], in0=psum[:, :], scalar1=b2_sb[:, :]
                )
                nc.sync.dma_start(out=out_ap[t], in_=fsb[::32, :])
    nc.compile()
    return nc


def _wrap_idx(a):
    """[E_PAD] int -> [128, NT*IDX_COLS] int16 in dma_gather's wrapped layout.

    Within tile t, index j (0..ET-1) sits at partition j%16 (replicated to all
    8 groups of 16 partitions), free column t*IDX_COLS + j//16.
    """
    m = a.reshape(NT, IDX_COLS, 16)          # [t, j//16, j%16]
    w = m.transpose(0, 2, 1)                 # [t, 16, IDX_COLS]
    w = np.tile(w, (1, 8, 1))                # [t, 128, IDX_COLS]
    w = w.transpose(1, 0, 2).reshape(128, NT * IDX_COLS)
    return np.ascontiguousarray(w, dtype=np.int16)


def get_nc():
    if "nc" not in _CACHE:
        _CACHE["nc"] = _build_nc()
    return _CACHE["nc"]


def make_in_maps(z_drug, z_reaction, row, col, W1, b1, W2, b2):
    f16 = np.float16
    zdT = np.zeros((H, A_ROWS), f16)
    zdT[:, :N_DRUG] = np.asarray(z_drug, np.float32).T.astype(f16)
    zrT = np.zeros((H, B_ROWS), f16)
    zrT[:, :N_REACTION] = np.asarray(z_reaction, np.float32).T.astype(f16)
    W1 = np.asarray(W1, np.float32)
    w1dT = np.ascontiguousarray(W1[:, :H].T).astype(f16)
    w1rT = np.ascontiguousarray(W1[:, H:].T).astype(f16)
    b1f = np.ascontiguousarray(
        np.broadcast_to(np.asarray(b1, np.float32).reshape(1, H), (128, H))
    )
    # w2T[p, c*32 + m] = W2[0, c*128 + p]  (chunk value replicated over 32 cols)
    w2c = np.asarray(W2, np.float32).reshape(KC, 128).T.astype(f16)  # [128, KC]
    w2T = np.ascontiguousarray(np.repeat(w2c[:, :, None], 32, axis=2).reshape(128, KC * 32))
    b2v = np.full((128, 1), float(np.asarray(b2).reshape(-1)[0]), np.float32)
    row = np.asarray(row).astype(np.int64)
    col = np.asarray(col).astype(np.int64)

    in_maps = []
    for ci in range(N_CORES):
        sl = slice(ci * E_CORE, (ci + 1) * E_CORE)
        r = np.zeros(E_PAD, np.int64)
        r[:E_CORE] = row[sl]
        c = np.zeros(E_PAD, np.int64)
        c[:E_CORE] = col[sl]
        in_maps.append(
            {
                "zdT": zdT,
                "zrT": zrT,
                "w1dT": w1dT,
                "w1rT": w1rT,
                "b1f": b1f,
                "w2T": w2T,
                "b2v": b2v,
                "rowidx": _wrap_idx(r),
                "colidx": _wrap_idx(c),
            }
        )
    return in_maps


def kernel(z_drug, z_reaction, row, col, W1, b1, W2, b2):
    from concourse.bass_utils import run_bass_kernel_spmd

    nc = get_nc()
    in_maps = make_in_maps(z_drug, z_reaction, row, col, W1, b1, W2, b2)
    res = run_bass_kernel_spmd(nc, in_maps, core_ids=list(range(N_CORES)))
    outs = [r["out"][:E_CORE] for r in res.results]
    return np.ascontiguousarray(np.concatenate(outs), dtype=np.float32)

